# revision 25
# baseline (speedup 1.0000x reference)
"""Trainium2 Bass kernel for the correlation-softargmax flow module.

Math (per batch b, query pixel q=(y,x)):
  c1 = l2norm_C(feature1), warp = l2norm_C(feature2)
  s[l,q] = <3x3 patch of warp at l, 3x3 patch of c1 at q>    (D = 32*9 = 288)
  p = softmax_l(10*s);  flow = (E_p[ix_l] - x, E_p[iy_l] - y)

Because softmax normalizes, only Z = sum_l exp, Sy = sum_l exp*iy, Sx = sum_l
exp*ix are needed per q (flash-attention style, no [L,L] materialization, and
exp(10*s - 30) needs no running max since |10*s| <= 90 and using a fixed shift
keeps everything in fp32 range).

Sharding: 8 cores = 4 batches x 2 query-row halves. Each core holds the full
K-side image (softmax runs over all 4096 l) and 2048 queries.

On-device layout: C=32 on partitions; zero-bordered [32,66,66] images stored
flat so each 3x3 tap is ONE contiguous 8.4KB-per-partition DMA run (the tap
shift is a flat source offset; two junk columns per row land in dest columns
64..65, which no matmul AP ever reads).  Taps are packed 3+3+3 into
96-partition d-major f16 patch tensors, so score matmuls run K=96.

The stats matmul (Z/Sy/Sx weights against exp(s)) uses an M=128-wide bf16
stationary slice from an overlapping [128,224] table even though only output
rows 0..2 are read.  This matters: the PE's HAM clock gate only reaches its
warm state (~2x clock: 216 ns per 512-row MM instead of 427 ns) when every MM
in the stream has a full-width stationary — one M=3 stats matmul per tile
pins the whole kernel at the cold rate (measured).  PSUM rows 3..127 of the
stats tile accumulate junk that is never read.
"""

import sys

import numpy as np

sys.path.insert(0, "/opt/trn_rl_repo")

import concourse.bass as bass  # noqa: E402
import concourse.mybir as mybir  # noqa: E402
import concourse.tile as tile  # noqa: E402
from concourse import bacc, bass_utils  # noqa: E402

F32 = mybir.dt.float32
F32R = mybir.dt.float32r
F16 = mybir.dt.float16
BF16 = mybir.dt.bfloat16

B, C, H, W = 4, 32, 64, 64
L = H * W              # 4096 match locations
NQ = L // 2            # queries per core
QROWS = H // 2         # query rows per core
N_CORES = 8
SCALE = 10.0
SHIFT = -30.0          # exp(10*s - 30): |10*s|<=90 so no overflow, and a row's
                       # max 10*s is never < -60 so Z stays far above underflow
EPS = 1e-12
WP = W + 2             # padded row width
TAPS = [(dy, dx) for dy in range(3) for dx in range(3)]

_NC_CACHE = {}
_LAST_RES = None


def _build_nc():
    nc = bacc.Bacc(None, target_bir_lowering=False)

    f1h = nc.dram_tensor("f1h", [C, QROWS + 2, W], F32, kind="ExternalInput")
    f2 = nc.dram_tensor("f2", [C, H, W], F32, kind="ExternalInput")
    w3 = nc.dram_tensor("w3", [128, 224], F32, kind="ExternalInput")
    yq4 = nc.dram_tensor("yq4", [128, 16], F32, kind="ExternalInput")
    xq4 = nc.dram_tensor("xq4", [128, 4], F32, kind="ExternalInput")
    outp = nc.dram_tensor("outp", [2, NQ], F32, kind="ExternalOutput")

    n1 = (QROWS + 2) * W   # 2176 pixels in the f1 halo slab
    np2 = (H + 2) * WP     # 4356 padded f2 pixels
    np1 = (QROWS + 2) * WP  # 2244 padded f1 pixels

    with tile.TileContext(nc) as tc:
        with tc.tile_pool(name="big", bufs=1) as big, \
             tc.tile_pool(name="work", bufs=1) as work, \
             tc.tile_pool(name="small", bufs=1) as small, \
             tc.tile_pool(name="pp", bufs=3) as pp, \
             tc.tile_pool(name="epi", bufs=2) as epi, \
             tc.tile_pool(name="nps", bufs=2, space="PSUM") as nps, \
             tc.tile_pool(name="sps", bufs=3, space="PSUM") as sps, \
             tc.tile_pool(name="stps", bufs=1, space="PSUM") as stps:

            # ---- constants ----
            onesf = small.tile([C, 1], F32, tag="onesf")
            nc.vector.memset(onesf, 1.0)
            ones32 = small.tile([C, 1], F32R, tag="ones32")
            nc.vector.tensor_copy(ones32, onesf)
            onesbf = small.tile([1, C], F32, tag="onesbf")
            nc.vector.memset(onesbf, 1.0)
            onesb = small.tile([1, C], F32R, tag="onesb")
            nc.vector.tensor_copy(onesb, onesbf)
            shiftc = small.tile([128, 1], F32, tag="shiftc")
            nc.vector.memset(shiftc, SHIFT)
            eps2c = small.tile([1, 1], F32, tag="eps2c")
            nc.vector.memset(eps2c, EPS * EPS)

            # ---- padded images, stored flat with 4 slack elems so the last
            # tap's contiguous read stays in bounds; only the 1-px border
            # needs zeroing (norm writes the interior) ----
            pad2 = big.tile([C, np2 + 4], F16, tag="pad2")
            pad1 = big.tile([C, np1 + 4], F16, tag="pad1")
            pad2v = pad2[:, 0:np2].rearrange("c (h w) -> c h w", w=WP)
            pad1v = pad1[:, 0:np1].rearrange("c (h w) -> c h w", w=WP)
            for pad, padv, nr, npx in ((pad2, pad2v, H + 2, np2),
                                       (pad1, pad1v, QROWS + 2, np1)):
                nc.vector.memset(padv[:, 0, :], 0.0)
                nc.vector.memset(padv[:, nr - 1, :], 0.0)
                nc.gpsimd.memset(padv[:, :, 0], 0.0)
                nc.gpsimd.memset(padv[:, :, W + 1], 0.0)
                nc.gpsimd.memset(pad[:, npx:npx + 4], 0.0)

            # ---- load inputs ----
            raw2 = big.tile([C, L], F32, tag="raw2")
            f2f = f2[:, :, :].rearrange("c h w -> c (h w)")
            nc.sync.dma_start(out=raw2[:, 0:L // 2], in_=f2f[:, 0:L // 2])
            nc.scalar.dma_start(out=raw2[:, L // 2:L], in_=f2f[:, L // 2:L])
            raw1 = big.tile([C, n1], F32, tag="raw1")
            f1f = f1h[:, :, :].rearrange("c h w -> c (h w)")
            nc.gpsimd.dma_start(out=raw1[:, 0:n1 // 2], in_=f1f[:, 0:n1 // 2])
            nc.sync.dma_start(out=raw1[:, n1 // 2:n1], in_=f1f[:, n1 // 2:n1])
            w3f = small.tile([128, 224], F32, tag="w3f")
            nc.sync.dma_start(out=w3f, in_=w3[:, :])
            w3r = small.tile([128, 224], BF16, tag="w3r")
            nc.vector.tensor_copy(w3r, w3f)
            xqs4 = small.tile([128, 4], F32, tag="xqs4")
            nc.sync.dma_start(out=xqs4, in_=xq4[:, :])
            yqs4 = small.tile([128, 16], F32, tag="yqs4")
            nc.sync.dma_start(out=yqs4, in_=yq4[:, :])
            # first gpsimd tensor op pays a ~3.5us library load; warm it now
            libw = small.tile([1, 8], F32, tag="libw")
            nc.gpsimd.tensor_mul(libw, onesbf[:, 0:8], onesbf[:, 0:8])

            # ---- l2 normalization over C, two stages so the two images can
            # interleave across engines.  C sits on partitions: the per-pixel
            # sum of squares comes from a ones-vector matmul, sqrt on scalar,
            # the reciprocal bounces through a [128, npix/128] layout (DVE
            # reciprocal is per-lane-serial), and the scaling multiply
            # broadcasts 1/norm back across partitions with a K=1 matmul. ----
            def norm_sumsq(raw, npix, img):
                sq = work.tile([C, npix], F32R, tag=f"sq{img}", name=f"sq{img}")
                quarter = npix // 4
                engs = [nc.vector, nc.gpsimd, nc.vector, nc.gpsimd]
                for k in range(4):
                    a, b = k * quarter, (k + 1) * quarter if k < 3 else npix
                    engs[k].tensor_mul(sq[:, a:b], raw[:, a:b], raw[:, a:b])
                nrow = work.tile([1, npix], F32, tag=f"nrow{img}",
                                 name=f"nrow{img}")
                nchunks = (npix + 511) // 512
                for j in range(nchunks):
                    n = min(512, npix - 512 * j)
                    ssp = nps.tile([1, 512], F32, tag="ssp", name="ssp")
                    nc.tensor.matmul(ssp[:, :n], ones32,
                                     sq[:, 512 * j:512 * j + n],
                                     start=True, stop=True)
                    # ln(ss); 1/norm later = exp(-0.5*ln(ss)).  Ln and Exp
                    # share one ACT table set (natural_log_exp_and_others),
                    # so the main loop's first Exp needs no table switch.
                    nc.scalar.activation(nrow[:, 512 * j:512 * j + n],
                                         ssp[:, :n],
                                         mybir.ActivationFunctionType.Ln,
                                         bias=eps2c)
                return nrow, nchunks

            def norm_apply(nTc, raw, npix, padv, row0, img):
                nrow, nchunks = nTc
                nT = work.tile([128, npix // 128], F32, tag=f"nT{img}",
                               name=f"nT{img}")
                nc.sync.dma_start(
                    out=nT, in_=nrow.rearrange("a (p c) -> a p c", p=128))
                rT2 = work.tile([128, npix // 128], F32R, tag=f"rT2{img}",
                                name=f"rT2{img}")
                nc.scalar.activation(rT2, nT,
                                     mybir.ActivationFunctionType.Exp,
                                     scale=-0.5)
                rrow = work.tile([1, npix], F32R, tag=f"rrow{img}",
                                 name=f"rrow{img}")
                nc.sync.dma_start(
                    out=rrow.rearrange("a (p c) -> a p c", p=128), in_=rT2)
                raw3 = raw.rearrange("c (h w) -> c h w", w=W)
                for j in range(nchunks):
                    n = min(512, npix - 512 * j)
                    rows = n // W  # chunks are whole image rows (512 = 8*64)
                    rb = nps.tile([C, 512], F32, tag="rb", name="rb")
                    nc.tensor.matmul(rb[:, :n], onesb,
                                     rrow[:, 512 * j:512 * j + n],
                                     start=True, stop=True)
                    r0 = 8 * j
                    nc.vector.tensor_mul(  # gpsimd cannot read PSUM
                        padv[:, row0 + r0:row0 + r0 + rows, 1:W + 1],
                        raw3[:, r0:r0 + rows, :],
                        rb[:, :n].rearrange("c (h w) -> c h w", w=W),
                    )

            nT2c = norm_sumsq(raw2, L, img=2)
            nT1c = norm_sumsq(raw1, n1, img=1)
            norm_apply(nT2c, raw2, L, pad2v, row0=1, img=2)

            # ---- d-major patch tensors: 3 groups of 3 taps (96 partitions).
            # K-side: l is enumerated in FLAT 66-wide padded coordinates
            # (l' = y*66 + x, 4224 slots, 33 l-tiles of 128), so every tap is
            # ONE contiguous 8.4KB-per-partition DMA run and every score
            # stationary is a contiguous 128-slice.  The 2 junk columns per
            # row (x=64,65) are zeroed -> their softmax weight is e^-30.
            # Q-side: the moving operand must be a single free dim, so qp
            # stays [96, 32, 64] contiguous via 9 strided tap copies. ----
            kp3 = [big.tile([96, H, WP], F16, tag=f"kp{g}", name=f"kp{g}")
                   for g in range(3)]
            # q-side: one 10-row tile per query block qt; partition block j
            # holds pad1 rows 8qt..8qt+9 shifted by dx=j, and the dy=g tap
            # window is just the contiguous row slice g..g+7.  Only 3 strided
            # copies per block (via gpsimd's software DGE; the hardware DGE
            # takes ~8.5ns per 128B run), and block 0 is ready ~2us after
            # pad1 so the main loop starts while later blocks stream in.
            qq = [big.tile([96, 10, W], F16, tag=f"qq{qt}", name=f"qq{qt}")
                  for qt in range(4)]
            kp_engs = [nc.sync, nc.scalar, nc.gpsimd]
            for t, (dy, dx) in enumerate(TAPS):
                g, j = divmod(t, 3)
                off = dy * WP + dx
                kp_engs[t % 3].dma_start(
                    out=kp3[g][32 * j:32 * j + 32, :, :].rearrange(
                        "p h w -> p (h w)"),
                    in_=pad2[:, off:off + H * WP])
            norm_apply(nT1c, raw1, n1, pad1v, row0=0, img=1)
            for qt in range(4):
                for j in range(3):
                    nc.gpsimd.dma_start(
                        out=qq[qt][32 * j:32 * j + 32, :, :],
                        in_=pad1v[:, 8 * qt:8 * qt + 10, j:j + W])
            # ~10 dummy score-shaped matmuls warm the HAM clock gate through
            # its 3.4us ramp while the qq copies land, so the real stream
            # starts at 2.4GHz.  They read kp3[0] so the scheduler cannot
            # hoist them ahead of the patch build into the cold prologue.
            kpf0 = kp3[0].rearrange("p h w -> p (h w)")
            for i in range(10):
                dps = sps.tile([128, 512], F32, tag="s", name="s")
                nc.tensor.matmul(dps, kpf0[:, 0:128], kpf0[:, 512:1024],
                                 start=True, stop=True)

            # ---- main loop: scores -> exp -> stats, flash-attention style.
            # Score MMs are [96,128]f16 x [96,512]f16; the stats MM is
            # [128,128]bf16 x [128,512]bf16 — both full-width-M so the PE
            # stream keeps the HAM gate warm. ----
            n_lt = (H * WP) // 128     # 33 flat-padded l-tiles
            n_qt = NQ // 512
            kpf = [kp3[g].rearrange("p h w -> p (h w)") for g in range(3)]
            qqf = [qq[qt].rearrange("p h w -> p (h w)") for qt in range(4)]
            for qt in range(n_qt):
                stats = stps.tile([128, 512], F32, tag="stats")
                # software-pipelined by two lt: the stats matmul for lt is
                # emitted after the score matmuls of lt+2, so the in-order PE
                # stream never waits on exp even with semaphore jitter
                pends = []
                for lt in range(n_lt):
                    s_ps = sps.tile([128, 512], F32, tag="s")
                    for g in range(3):
                        nc.tensor.matmul(
                            s_ps,
                            kpf[g][:, 128 * lt:128 * lt + 128],
                            qqf[qt][:, W * g:W * g + 512],
                            start=(g == 0), stop=(g == 2),
                        )
                    if lt >= 2:
                        lp = lt - 2
                        nc.tensor.matmul(
                            stats, w3r[:, 3 * lp:3 * lp + 128],
                            pends[lp], start=(lp == 0), stop=False)
                    p_sb = pp.tile([128, 512], BF16, tag="p")
                    nc.scalar.activation(p_sb, s_ps,
                                         mybir.ActivationFunctionType.Exp,
                                         bias=shiftc, scale=SCALE)
                    pends.append(p_sb)
                for lp in range(n_lt - 2, n_lt):
                    nc.tensor.matmul(stats, w3r[:, 3 * lp:3 * lp + 128],
                                     pends[lp], start=False,
                                     stop=(lp == n_lt - 1))

                # flow = S/Z - coord.  Stats rows go PSUM -> [1,1536] row ->
                # per-stat [128,4] blocks (q = 512*qt + 4*p + c) so the
                # reciprocal and flow math run 128 lanes wide.
                st3 = epi.tile([3, 512], F32, tag="st3")
                nc.vector.tensor_copy(st3, stats[0:3, :])
                zsb = epi.tile([1, 3 * 512], F32, tag="zsb")
                nc.sync.dma_start(out=zsb.rearrange("a (b c) -> a b c", c=512),
                                  in_=st3)
                sT = epi.tile([128, 12], F32, tag="sT")
                for r in range(3):
                    nc.sync.dma_start(
                        out=sT[:, 4 * r:4 * r + 4],
                        in_=zsb[:, 512 * r:512 * r + 512].rearrange(
                            "a (p c) -> a p c", p=128))
                rz4 = epi.tile([128, 4], F32, tag="rz4")
                nc.vector.reciprocal(rz4, sT[:, 0:4])
                fw4 = epi.tile([128, 4], F32, tag="fw4")
                nc.vector.tensor_mul(fw4, sT[:, 8:12], rz4)
                nc.vector.tensor_sub(fw4, fw4, xqs4)
                fh4 = epi.tile([128, 4], F32, tag="fh4")
                nc.vector.tensor_mul(fh4, sT[:, 4:8], rz4)
                nc.vector.tensor_sub(fh4, fh4, yqs4[:, 4 * qt:4 * qt + 4])
                nc.sync.dma_start(
                    out=outp[0:1, 512 * qt:512 * qt + 512].rearrange(
                        "a (p c) -> a p c", p=128), in_=fw4)
                nc.sync.dma_start(
                    out=outp[1:2, 512 * qt:512 * qt + 512].rearrange(
                        "a (p c) -> a p c", p=128), in_=fh4)

    nc.finalize()
    return nc


def _host_consts():
    p = np.arange(128)
    w3 = np.zeros((128, 224), np.float32)
    for t in range(33):
        lflat = 128 * t + p                  # flat-padded l' = 66*y + x
        y, x = lflat // 66, lflat % 66
        valid = (x < 64) & (y < 64)
        # junk flat-padded columns (x>=64) get weight 0 in ALL stat rows, so
        # their (garbage) exp values contribute exactly nothing -- no need to
        # zero the kp junk columns on device
        w3[:, 3 * t] = np.where(valid, 1.0, 0.0)
        w3[:, 3 * t + 1] = np.where(valid, y, 0.0)
        w3[:, 3 * t + 2] = np.where(valid, x, 0.0)
    # q = 512*qt + 4*p + c:  x = q%64 = (4p+c)%64 ; y = q//64 = 8qt+(4p+c)//64
    pc = 4 * p[:, None] + np.arange(4)[None, :]      # [128, 4]
    xq4 = (pc % 64).astype(np.float32)
    yq4 = np.zeros((128, 16), np.float32)
    for qt in range(4):
        yq4[:, 4 * qt:4 * qt + 4] = 8 * qt + pc // 64
    return w3, xq4, yq4


def kernel(feature1, feature2):
    feature1 = np.ascontiguousarray(feature1, np.float32)
    feature2 = np.ascontiguousarray(feature2, np.float32)
    w3, xq4, yq4 = _host_consts()

    f1p = np.zeros((B, C, H + 2, W), np.float32)
    f1p[:, :, 1:H + 1, :] = feature1

    in_maps = []
    for core in range(N_CORES):
        b, h = divmod(core, 2)
        in_maps.append({
            "f1h": np.ascontiguousarray(f1p[b, :, h * QROWS:h * QROWS + QROWS + 2, :]),
            "f2": np.ascontiguousarray(feature2[b]),
            "w3": w3,
            "yq4": yq4 + h * QROWS,
            "xq4": xq4,
        })

    if "nc" not in _NC_CACHE:
        _NC_CACHE["nc"] = _build_nc()
    res = bass_utils.run_bass_kernel_spmd(
        _NC_CACHE["nc"], in_maps, core_ids=list(range(N_CORES)))
    global _LAST_RES
    _LAST_RES = res

    out = np.zeros((B, 2, H, W), np.float32)
    for core in range(N_CORES):
        b, h = divmod(core, 2)
        out[b, :, h * QROWS:(h + 1) * QROWS, :] = (
            res.results[core]["outp"].reshape(2, QROWS, W))
    return out
